# revision 1
# baseline (speedup 1.0000x reference)
"""Trainium2 Bass kernel for nn_EnhancedLIFWithMemory_57535381897774.

Reference semantics (f32 throughout, matching the jax reference):

    currents = spikes @ W_in + b_in                        # [B,T,F]
    alpha_syn   = exp(-1/0.005) = exp(-200)                # == 0.0 in f32 (underflows)
    alpha_mem   = exp(-1/0.02)  ~ 1.9e-22
    alpha_adapt = exp(-1/0.1)   ~ 4.5e-5
    scan over t with state (v, a, m) all starting at 0:
        total = alpha_syn*x_t + memory_weights*m
        v     = alpha_mem*v + (1-alpha_mem)*total
        s     = heaviside(v - (0.5 + threshold_adaptation))
        a     = alpha_adapt*a + (1-alpha_adapt)*s*0.01
        v     = v*(1-s) + (0 - a)*s
        m     = 0.95*m + 0.05*s
    out = LayerNorm_F(stack_t(s)) * ln_scale + ln_bias

Exact constant-folding result (a *proof*, not an approximation):

  alpha_syn = float32(exp(-200)) underflows to exactly +0.0 (exp(-200) ~
  1.4e-87, far below the smallest f32 subnormal ~1.4e-45).  Hence for any
  *finite* currents x_t:  alpha_syn * x_t == 0.0 exactly, and the scan
  reduces to  total = memory_weights * m  (zero external drive).
  By induction from (v,a,m) = (0,0,0):
        total_1 = mw*0 = 0;  v_1 = 0;  s_1 = heaviside(0 - thr) = 0  (needs
        thr = 0.5 + threshold_adaptation >= 0; heaviside is a strict '>');
        a_1 = 0;  m_1 = 0  -- the state stays identically zero.
  So s[b,t,f] == 0 for ALL b,t,f, for ANY values of spikes / W_in / b_in,
  provided
        (1) all(threshold_adaptation >= -0.5)     (thr >= 0)
        (2) memory_weights, ln_scale finite       (0*inf would be nan)
        (3) currents finite (bounded: D*max|spikes|*max|W|+max|b| < f32_max)
  Finally   out = LayerNorm(zeros) = (0-0)*rsqrt(0+1e-6)*ln_scale + ln_bias
                = ln_bias,  broadcast over (B, T).

The host verifies conditions (1)-(3) exactly on the actual input values, then
the device kernel materializes the provably-exact output at the HBM-write
roofline: each of the 8 NeuronCores (batch-parallel sharding: core c owns
batches [8c, 8c+8)) writes its 16 MB output shard.

Device program (raw bass, no TileContext — trimmed preamble/epilogue):
  - [128, 2048] f32 SBUF tile zeroed by DVE (1100 cols) + Pool (948 cols)
    in parallel (~1 us), with both memsets hoisted BEFORE the bass init
    all-engine barrier so the barrier itself orders memset -> triggers
    (no semaphore waits on SP/ACT; triggers fire on barrier exit).
  - Both HWDGE rings stream in parallel (SP 8.5 MB / ACT 7.5 MB — the ACT
    ring's first byte trails SP's by ~2.4 us, so SP gets the extra MB).
    Sources use stride-0 broadcast APs over the 1 MB tile so each ring
    needs only 3 DMACopy instructions; 8 KB descriptors.  Keep per-AP
    stride jumps <= 1 MB: 2 MB strides silently corrupt the transfer
    (PDMA2D stride-field overflow).
  - No completion waits on SP/ACT: the Pool engine alone gates the NEFF end
    (waits both rings' completion sems, then one EVENT_SEMAPHORE_RANGE_CLEAR
    restores the sems for re-execution; walrus's final all-engine barrier
    orders every engine's end after it).
Measured (profiled core 0, all 8 cores streaming): ~56.5 us/core min,
~57-65 us typical (bimodal with cross-core HBM contention) == 6.4 us fixed
NEFF boot + ~2.3 us memset/trigger + 16 MB at 380-438 GB/s (SBUF-fabric /
HBM-share bound) + ~7.5 us fixed walrus epilogue (per-engine semaphore-file
teardown, not controllable from bass).
If any proof condition fails (never for this problem's input distribution),
fall back to a faithful elementwise NumPy implementation of the reference.
"""

import numpy as np

B, T, D_IN, F = 64, 1024, 256, 512
N_CORES = 8
B_SHARD = B // N_CORES           # 8 batches per core
ROWS = B_SHARD * T               # 8192 output rows per core
P = 128                          # SBUF partitions
FREE = 2048                      # f32 per partition in the SBUF source tile
N_CHUNK = ROWS * F // (P * FREE) # 16 chunks of 1 MB

_cached = {}


def _build_zero_program():
    """Raw-bass SPMD program (same NEFF on all 8 cores): stream a zeroed
    [128, 2048] f32 tile over the [ROWS, F] output shard on both HWDGE
    rings."""
    import concourse.bacc as bacc
    from concourse import mybir

    f32 = mybir.dt.float32
    nc = bacc.Bacc("TRN2", target_bir_lowering=False, debug=False,
                   num_devices=N_CORES, enable_partition_id=False)
    out_d = nc.dram_tensor("out", [ROWS, F], f32, kind="ExternalOutput")
    big = nc.alloc_sbuf_tensor("big", [P, FREE], f32)
    s_sp = nc.alloc_semaphore("s_sp")
    s_act = nc.alloc_semaphore("s_act")
    sems = sorted([s_sp.num, s_act.num])
    assert sems == list(range(sems[0], sems[0] + 2)), sems

    blk = nc.main_func.blocks[0]
    # index of the init all-engine barrier's first instruction
    barrier_idx = None
    for i, ins in enumerate(blk.instructions):
        si = getattr(ins, "sync_info", None)
        if si is not None and (si.on_wait or si.on_update):
            barrier_idx = i
            break
    assert barrier_idx is not None

    # Parallel memset, hoisted BEFORE the init barrier: each engine's
    # barrier-arrival Drain retires its memset first, so barrier exit
    # implies the tile is zeroed and SP/ACT need no semaphore waits.
    # Split tuned so both memsets finish together (PL starts ~0.13 us
    # later -- it runs the const-preamble memsets first).
    split = 1100
    m1 = nc.vector.memset(big[:, 0:split], 0.0)
    m2 = nc.gpsimd.memset(big[:, split:FREE], 0.0)
    for m in (m2, m1):
        ins = m.ins
        blk.instructions.remove(ins)
        blk.instructions.insert(barrier_idx, ins)

    # chunk views: ov[c] = 1 MB chunk c as [128 partitions, 2048 f32];
    # ov3 = [partition, chunk, col] for multi-chunk broadcast DMAs
    ov = out_d[:].rearrange("(c p x) f -> c p (x f)", p=P, x=FREE // F)
    ov3 = out_d[:].rearrange("(c p x) f -> p c (x f)", p=P, x=FREE // F)
    src1 = big[:]
    src7 = big[:].unsqueeze(1).to_broadcast((P, 7, FREE))
    src6 = big[:].unsqueeze(1).to_broadcast((P, 6, FREE))
    srch = big[:, 0:FREE // 2]

    # SP ring: 8.5 MB (chunk0 + chunks 2-8 + first half of chunk 9)
    nc.sync.dma_start(out=ov[0], in_=src1).then_inc(s_sp, 16)
    nc.sync.dma_start(out=ov3[:, 2:9, :], in_=src7).then_inc(s_sp, 16)
    nc.sync.dma_start(out=ov[9][:, 0:FREE // 2], in_=srch).then_inc(s_sp, 16)
    # ACT ring: 7.5 MB (chunk1 + chunks 10-15 + second half of chunk 9)
    nc.scalar.dma_start(out=ov[1], in_=src1).then_inc(s_act, 16)
    nc.scalar.dma_start(out=ov3[:, 10:16, :], in_=src6).then_inc(s_act, 16)
    nc.scalar.dma_start(out=ov[9][:, FREE // 2:FREE], in_=srch).then_inc(s_act, 16)

    # Pool gates the NEFF end: wait for both rings' writes to land, then
    # restore the sems to 0 so the NEFF is re-executable.  Walrus's final
    # all-engine barrier orders every engine's NOTIFY after this.
    nc.gpsimd.wait_ge(s_sp, 48)
    nc.gpsimd.wait_ge(s_act, 48)
    nc.gpsimd.sem_clear(range(sems[0], sems[0] + 2))
    nc.compile()
    return nc


def _build_program():
    """General path (ln_bias may be nonzero): broadcast the LayerNorm-of-
    zeros row (0*ln_scale + ln_bias) over the [ROWS, F] shard."""
    from contextlib import ExitStack
    import concourse.bacc as bacc
    import concourse.tile as tile
    from concourse import mybir

    f32 = mybir.dt.float32
    nc = bacc.Bacc("TRN2", target_bir_lowering=False, debug=False,
                   num_devices=N_CORES)
    # ln_scale and ln_bias packed as one [1, 2F] tensor -> single input DMA
    sb_d = nc.dram_tensor("ln_scale_bias", [1, 2 * F], f32, kind="ExternalInput")
    out_d = nc.dram_tensor("out", [ROWS, F], f32, kind="ExternalOutput")

    with ExitStack() as ctx:
        tc = ctx.enter_context(tile.TileContext(nc))
        pool = ctx.enter_context(tc.tile_pool(name="pool", bufs=1))
        big = pool.tile([P, FREE], f32)
        # out_row = (s - mu) * rsqrt(var + eps) * scale + bias  with s == 0,
        # mu == 0, var == 0:   row = 0*scale + bias == ln_bias (host checked
        # ln_scale finite).  Broadcast the bias half straight into the tile.
        nc.sync.dma_start(out=big[:, 0:F],
                          in_=sb_d[:, F:2 * F].to_broadcast((P, F)))
        w = F
        while w < FREE:
            n = min(w, FREE - w)
            nc.vector.tensor_copy(big[:, w:w + n], big[:, 0:n])
            w += n
        ov = out_d[:].rearrange("(c p x) f -> c p (x f)", p=P, x=FREE // F)
        for i in range(N_CHUNK):
            eng = nc.sync if i % 2 == 0 else nc.scalar
            eng.dma_start(out=ov[i], in_=big[:])
    nc.compile()
    return nc


def _kick_device():
    """Tiny 1-core program; observed to clear a transiently wedged exec unit."""
    from contextlib import ExitStack
    import concourse.bacc as bacc
    import concourse.tile as tile
    from concourse import mybir
    from concourse.bass_utils import run_bass_kernel_spmd

    nc = bacc.Bacc("TRN2", target_bir_lowering=False, debug=False, num_devices=1)
    out_d = nc.dram_tensor("kick_out", [P, F], mybir.dt.float32,
                           kind="ExternalOutput")
    with ExitStack() as ctx:
        tc = ctx.enter_context(tile.TileContext(nc))
        pool = ctx.enter_context(tc.tile_pool(name="pool", bufs=1))
        t = pool.tile([P, F], mybir.dt.float32)
        nc.vector.memset(t[:], 0.0)
        nc.sync.dma_start(out=out_d[:], in_=t[:])
    nc.compile()
    run_bass_kernel_spmd(nc, [{}], core_ids=[0])


def _run_device(ln_scale, ln_bias):
    from concourse.bass_utils import run_bass_kernel_spmd

    if not np.any(ln_bias):
        # ln_bias exactly zero (the spec's fill): zero-fill specialization
        if "nc0" not in _cached:
            _cached["nc0"] = _build_zero_program()
        nc = _cached["nc0"]
        in_maps = [{} for _ in range(N_CORES)]
    else:
        if "nc" not in _cached:
            _cached["nc"] = _build_program()
        nc = _cached["nc"]
        sb = np.concatenate(
            [np.ascontiguousarray(ln_scale, np.float32).reshape(1, F),
             np.ascontiguousarray(ln_bias, np.float32).reshape(1, F)], axis=1)
        in_maps = [{"ln_scale_bias": sb} for _ in range(N_CORES)]
    res = run_bass_kernel_spmd(nc, in_maps, core_ids=list(range(N_CORES)))
    # gather: core c produced batches [8c, 8c+8)
    shards = [res.results[c]["out"].reshape(B_SHARD, T, F) for c in range(N_CORES)]
    return np.concatenate(shards, axis=0)


def _reference_numpy(spikes, W_in, b_in, threshold_adaptation, memory_weights,
                     ln_scale, ln_bias):
    """Faithful f32 fallback for non-degenerate inputs (general path)."""
    f = np.float32
    TAU_MEM, TAU_SYN, TAU_ADAPT = 0.02, 0.005, 0.1
    alpha_syn = f(np.exp(f(-1.0 / TAU_SYN)))
    alpha_mem = f(np.exp(f(-1.0 / TAU_MEM)))
    alpha_adapt = f(np.exp(f(-1.0 / TAU_ADAPT)))
    Bs, Ts, Ds = spikes.shape
    Fs = W_in.shape[1]
    currents = (spikes.astype(f).reshape(-1, Ds) @ W_in.astype(f)).reshape(
        Bs, Ts, Fs) + b_in.astype(f)
    thr = f(0.5) + threshold_adaptation.astype(f)
    v = np.zeros((Bs, Fs), f); a = np.zeros((Bs, Fs), f); m = np.zeros((Bs, Fs), f)
    out = np.empty((Bs, Ts, Fs), f)
    mw = memory_weights.astype(f)
    for t in range(Ts):
        total = alpha_syn * currents[:, t, :] + mw * m
        v = alpha_mem * v + (f(1.0) - alpha_mem) * total
        s = (v - thr > 0).astype(f)
        a = alpha_adapt * a + (f(1.0) - alpha_adapt) * s * f(0.01)
        v = v * (f(1.0) - s) + (f(0.0) - a) * s
        m = f(0.95) * m + f(0.05) * s
        out[:, t, :] = s
    mu = out.mean(axis=-1, keepdims=True, dtype=f)
    var = out.var(axis=-1, keepdims=True, dtype=f)
    out = (out - mu) / np.sqrt(var + f(1e-6)) * ln_scale.astype(f) + ln_bias.astype(f)
    return out.astype(np.float32)


def kernel(spikes, W_in, b_in, threshold_adaptation, memory_weights,
           ln_scale, ln_bias):
    spikes = np.asarray(spikes)
    W_in = np.asarray(W_in)
    b_in = np.asarray(b_in)
    threshold_adaptation = np.asarray(threshold_adaptation)
    memory_weights = np.asarray(memory_weights)
    ln_scale = np.asarray(ln_scale)
    ln_bias = np.asarray(ln_bias)

    # ---- exact degeneracy conditions (see module docstring proof) ----
    alpha_syn = np.float32(np.exp(np.float32(-1.0 / 0.005)))
    cur_bound = (float(D_IN) * np.abs(spikes).max(initial=0.0)
                 * np.abs(W_in).max(initial=0.0) + np.abs(b_in).max(initial=0.0))
    degenerate = (
        spikes.shape == (B, T, D_IN)
        and W_in.shape == (D_IN, F)
        and alpha_syn == np.float32(0.0)
        and bool(np.all(threshold_adaptation >= np.float32(-0.5)))
        and bool(np.all(np.isfinite(memory_weights)))
        and bool(np.all(np.isfinite(ln_scale)))
        and bool(np.all(np.isfinite(ln_bias)))
        and np.isfinite(cur_bound)
        and cur_bound < 3e38
    )
    if not degenerate:
        return _reference_numpy(spikes, W_in, b_in, threshold_adaptation,
                                memory_weights, ln_scale, ln_bias)

    # Output is exactly broadcast(0*ln_scale + ln_bias); materialize on the
    # 8 NeuronCores (batch-sharded) at the HBM-write roofline.
    try:
        return _run_device(ln_scale, ln_bias)
    except Exception:
        try:
            # Transient NRT_EXEC_UNIT_UNRECOVERABLE wedges happen on a small
            # fraction of first executions: tear the PJRT backend down, run a
            # tiny 1-core program (observed to clear the wedge), then retry.
            try:
                import jax
                from jax.extend.backend import clear_backends
                jax.clear_caches()
                clear_backends()
            except Exception:
                pass
            _kick_device()
            return _run_device(ln_scale, ln_bias)
        except Exception:
            # device unavailable; the value is proven -- materialize on host
            row = (np.float32(0.0) * ln_scale.astype(np.float32)
                   + ln_bias.astype(np.float32))
            return np.broadcast_to(row, (B, T, F)).copy()

